# revision 1
# baseline (speedup 1.0000x reference)
"""FlowNetC correlation (max_displacement=20, stride2=2) on 8 trn2 NeuronCores.

Strategy: data-parallel over batch (B=8 -> 1 batch/core). Per core, the
cost volume out[d,y,x] = (1/C) sum_c in1[c,y,x]*in2p[c,y+oy,x+ox] is
computed as a banded Gram matrix on the tensor engine:

  - stationary (lhsT): 128 in1 feature vectors at an 8x16 grid of
    positions (y0+2i, x0+2j)  [one (y,x)-parity class, spacing 2 to
    match the displacement stride]
  - moving (rhs): in2p feature vectors over the 28x36 window
    (y0+2kr, x0+2ks), kr<28, ks<36
  - psum[m=(i,j), n=(kr,ks)] = dot(in1_m, in2p_n); the 441 useful
    displacements for position m sit at kr in [i,i+20], ks in [j,j+20].

The slightly-overcomplete [y,x,28*36] tensor is written to DRAM with
fully-regular access patterns (the band extraction is an inherently
per-partition shear no engine/DMA AP can express); the final 21x21
window slice per position happens on host in numpy. All device I/O is
bf16 (inputs pre-scaled by 1/C=2^-8 exactly on host; psum accumulates
in fp32).
"""

import numpy as np
import ml_dtypes

B, C, H, W = 8, 256, 96, 128
MAXD = 20  # pad size
PH, PW = H + 2 * MAXD, W + 2 * MAXD  # 136, 168
A_, B_ = 16, 8  # stationary grid (rows i, cols j)
KR, KS = A_ + MAXD, B_ + MAXD  # 28, 36 moving window
NF = KR * KS  # 1008 psum free size
N_CORES = 8

_cached = None


def _split_multiwait(nc):
    """This walrus build accepts at most one sem-wait per instruction.
    Move extra waits onto standalone EventSemaphore carriers inserted
    just before the instruction (same engine => program order holds)."""
    import concourse.mybir as mybir

    n = 0
    for f in nc.m.functions:
        for bb in f.blocks:
            insts = bb.instructions
            i = 0
            while i < len(insts):
                inst = insts[i]
                si = inst.sync_info
                if si is not None and si.on_wait and len(si.on_wait) > 1:
                    waits = list(si.on_wait)
                    si.on_wait = waits[-1:]
                    for w in waits[:-1]:
                        car = mybir.InstEventSemaphore(
                            name=f"WSPLIT-{n}", ins=[], outs=[]
                        )
                        n += 1
                        car.engine = inst.engine
                        car.sync_info = type(si)(on_wait=[w], on_update=[])
                        insts.insert(i, car)
                        i += 1
                i += 1
    return n


def _build():
    import concourse.bass as bass
    import concourse.mybir as mybir
    import concourse.tile as tile

    bf16 = mybir.dt.bfloat16
    f32 = mybir.dt.float32

    nc = bass.Bass("TRN2", target_bir_lowering=False, debug=False)
    # x1 arrives host-packed: [c, tile, m] with tile=(ty,tx,py,px), m=(i,j)
    x1 = nc.dram_tensor("x1", [C, H, W], bf16, kind="ExternalInput").ap()
    x2 = nc.dram_tensor("x2", [C, H, W], bf16, kind="ExternalInput").ap()
    z = nc.dram_tensor("z", [H, W, NF], bf16, kind="ExternalOutput").ap()

    with tile.TileContext(nc) as tc:
        with (
            tc.tile_pool(name="resident", bufs=1) as rpool,
            tc.tile_pool(name="psum", bufs=4, space="PSUM") as ppool,
            tc.tile_pool(name="s2", bufs=6) as spool,
        ):
            a_sb = []  # in1 chunks [128, H*W]
            p_sb = []  # padded in2 chunks [128, PH*PW]
            for k in range(2):
                a = rpool.tile([128, H * W], bf16, tag=f"a{k}")
                a_sb.append(a)
                p = rpool.tile([128, PH * PW], bf16, tag=f"p{k}")
                p_sb.append(p)

            for k in range(2):
                p3 = p_sb[k][:].rearrange("p (r s) -> p r s", r=PH, s=PW)
                # zero borders: top rows, bottom rows, left/right cols
                nc.vector.memset(p3[:, 0:MAXD, :], 0.0)
                nc.vector.memset(p3[:, PH - MAXD : PH, :], 0.0)
                nc.vector.memset(p3[:, MAXD : PH - MAXD, 0:MAXD], 0.0)
                nc.vector.memset(p3[:, MAXD : PH - MAXD, PW - MAXD : PW], 0.0)
            # band-split interior loads so early tiles' matmuls only wait
            # on the first bands; alternate the two HWDGE rings
            NB = 6
            hb = H // NB
            for b in range(NB):
                for k in range(2):
                    p3 = p_sb[k][:].rearrange("p (r s) -> p r s", r=PH, s=PW)
                    eng = nc.sync if (b + k) % 2 == 0 else nc.scalar
                    eng.dma_start(
                        p3[:, MAXD + b * hb : MAXD + (b + 1) * hb, MAXD : MAXD + W],
                        x2[k * 128 : (k + 1) * 128, b * hb : (b + 1) * hb, :],
                    )
                    eng2 = nc.scalar if (b + k) % 2 == 0 else nc.sync
                    eng2.dma_start(
                        a_sb[k][:, b * (hb * W) : (b + 1) * (hb * W)],
                        x1[k * 128 : (k + 1) * 128, b * hb : (b + 1) * hb, :].rearrange(
                            "c h w -> c (h w)"
                        ),
                    )

            # views for parity-strided slicing
            p_v = [
                p_sb[k][:].rearrange(
                    "p (rq rp sq sp) -> p rq rp sq sp", rp=2, sp=2, sq=PW // 2
                )
                for k in range(2)
            ]
            z_v = z.rearrange(
                "(yq yp) (xq xp) n -> yq yp xq xp n", yp=2, xp=2
            )

            t_idx = 0
            for ty in range(H // (2 * A_)):
                for tx in range(W // (2 * B_)):
                    for py in range(2):
                        for px in range(2):
                            # stationary grid rows y=16ty+py+2i, cols x=32tx+px+2j
                            lhs = [
                                a_sb[k][:, t_idx * 128 : (t_idx + 1) * 128]
                                for k in range(2)
                            ]
                            t_idx += 1
                            ps = [
                                ppool.tile([128, NF // 2], f32, name=f"ps{h}", tag=f"ps{h}")
                                for h in range(2)
                            ]
                            for k in range(2):
                                for h in range(2):
                                    rhs = p_v[k][
                                        :,
                                        A_ * ty + (KR // 2) * h : A_ * ty + (KR // 2) * (h + 1),
                                        py,
                                        B_ * tx : B_ * tx + KS,
                                        px,
                                    ]
                                    nc.tensor.matmul(
                                        ps[h][:],
                                        lhs[k],
                                        rhs,
                                        start=(k == 0),
                                        stop=(k == 1),
                                    )
                            s2 = spool.tile([128, NF], bf16)
                            nc.vector.tensor_copy(s2[:, 0 : NF // 2], ps[0][:])
                            nc.scalar.copy(s2[:, NF // 2 : NF], ps[1][:])
                            out_eng = nc.sync if t_idx % 2 == 0 else nc.scalar
                            out_eng.dma_start(
                                z_v[
                                    A_ * ty : A_ * ty + A_,
                                    py,
                                    B_ * tx : B_ * tx + B_,
                                    px,
                                    :,
                                ],
                                s2[:],
                            )

    _split_multiwait(nc)
    return nc


def kernel(input1, input2):
    global _cached
    from concourse import bass_utils

    if _cached is None:
        _cached = _build()
    nc = _cached

    # exact 1/C scale (2^-8) folded into in1 before the bf16 rounding
    x1 = (input1 * np.float32(1.0 / C)).astype(ml_dtypes.bfloat16)
    # pack stationary tiles contiguously: [c, (ty,tx,py,px), (i,j)]
    x1 = np.ascontiguousarray(
        x1.reshape(B, C, H // (2 * A_), A_, 2, W // (2 * B_), B_, 2).transpose(0, 1, 2, 5, 4, 7, 3, 6)
    ).reshape(B, C, H, W)
    x2 = input2.astype(ml_dtypes.bfloat16)
    in_maps = [{"x1": x1[b], "x2": x2[b]} for b in range(N_CORES)]
    res = bass_utils.run_bass_kernel_spmd(
        nc, in_maps, core_ids=list(range(N_CORES))
    )
    Z = np.stack([res.results[b]["z"] for b in range(N_CORES)])
    Zf = Z.astype(np.float32).reshape(B, H, W, KR, KS)

    D = 21
    out = np.empty((B, D * D, H, W), np.float32)
    ystep, xstep = 2 * A_, 2 * B_
    for yy in range(ystep):
        i = yy // 2
        for xx in range(xstep):
            j = xx // 2
            blk = Zf[:, yy::ystep, xx::xstep, i : i + D, j : j + D]
            out[:, :, yy::ystep, xx::xstep] = blk.reshape(
                B, H // ystep, W // xstep, D * D
            ).transpose(0, 3, 1, 2)
    return out



# revision 2
# speedup vs baseline: 1.2573x; 1.2573x over previous
"""FlowNetC correlation (max_displacement=20, stride2=2) on 8 trn2 NeuronCores.

v3: banded-Gram tensor-engine kernel, 1 batch/core.
  - inputs loaded with huge contiguous descriptors (11-13KB/partition) in
    dependency order (x1 tile-band 0, then x2 rows [0,52) both chunks, then
    x2 rows [52,96), then x1 remainder) so the PE stream starts ~10us in
    and never data-stalls after.
  - z written per (ty,tx) quad of tiles with a 4-step i-staircase: psum
    cols for stationary rows i in [4g,4g+4) lie in [112g, 112g+672) ->
    four plain partition-aligned DMAs per quad write 1.52x-overcomplete
    cost volume instead of 2.29x.
  - y-padding on device (memset on idle gpsimd); x-out-of-range
    displacements read wrapped junk that the host masks to zero.
"""

import numpy as np
import ml_dtypes

B, C, H, W = 8, 256, 96, 128
MAXD = 20
A_, B_ = 16, 8
KR, KS = 36, 28
NF = KR * KS  # 1008
PH = H + 2 * MAXD  # 136
MARG = 24
ENDM = 24
PSZ = MARG + PH * W + ENDM
N_CORES = 8
NT = 96  # tiles of 128 positions
RSPLIT = 52  # x2 row split point

_cached = None


def _split_multiwait(nc):
    """Walrus accepts at most one sem-wait per instruction; move extras onto
    standalone EventSemaphore carriers inserted just before (same engine)."""
    import concourse.mybir as mybir

    n = 0
    for f in nc.m.functions:
        for bb in f.blocks:
            insts = bb.instructions
            i = 0
            while i < len(insts):
                inst = insts[i]
                si = inst.sync_info
                if si is not None and si.on_wait and len(si.on_wait) > 1:
                    waits = list(si.on_wait)
                    si.on_wait = waits[-1:]
                    for w in waits[:-1]:
                        car = mybir.InstEventSemaphore(
                            name=f"WSPLIT-{n}", ins=[], outs=[]
                        )
                        n += 1
                        car.engine = inst.engine
                        car.sync_info = type(si)(on_wait=[w], on_update=[])
                        insts.insert(i, car)
                        i += 1
                i += 1
    return n


def _build():
    import concourse.bass as bass
    import concourse.mybir as mybir
    import concourse.tile as tile
    from concourse.ap import AP

    bf16 = mybir.dt.bfloat16
    f32 = mybir.dt.float32

    nc = bass.Bass("TRN2", target_bir_lowering=False, debug=False)
    x1 = nc.dram_tensor("x1", [C, H, W], bf16, kind="ExternalInput").ap()
    x2 = nc.dram_tensor("x2", [C, H, W], bf16, kind="ExternalInput").ap()
    # z: [quad(ty,tx), g, part32, tile4(py,px), 672]
    z = nc.dram_tensor("z", [NT // 4, 4, 32, 4, 672], bf16, kind="ExternalOutput").ap()

    with tile.TileContext(nc) as tc:
        with (
            tc.tile_pool(name="resident", bufs=1) as rpool,
            tc.tile_pool(name="psum", bufs=4, space="PSUM") as ppool,
            tc.tile_pool(name="s2", bufs=8) as spool,
        ):
            a_sb = []
            p_sb = []
            for k in range(2):
                a_sb.append(rpool.tile([128, H * W], bf16, name=f"a{k}", tag=f"a{k}"))
                p_sb.append(rpool.tile([128, PSZ], bf16, name=f"p{k}", tag=f"p{k}"))

            # pad memsets split across engines so none serializes the start
            nc.vector.memset(p_sb[0][:, 0 : MARG + MAXD * W], 0.0)
            nc.gpsimd.memset(p_sb[0][:, MARG + (MAXD + H) * W : PSZ], 0.0)
            nc.gpsimd.memset(p_sb[1][:, 0 : MARG + MAXD * W], 0.0)
            nc.vector.memset(p_sb[1][:, MARG + (MAXD + H) * W : PSZ], 0.0)

            # loads: dependency-ordered, huge contiguous descriptors
            def ld_x1(eng, k, c0, c1):
                eng.dma_start(
                    a_sb[k][:, c0:c1],
                    x1[k * 128 : (k + 1) * 128].rearrange("c h w -> c (h w)")[:, c0:c1],
                )

            def ld_x2(eng, k, r0, r1):
                eng.dma_start(
                    p_sb[k][:, MARG + (MAXD + r0) * W : MARG + (MAXD + r1) * W],
                    x2[k * 128 : (k + 1) * 128, r0:r1, :].rearrange("c h w -> c (h w)"),
                )

            # consumption-ordered: x2 row pieces sized to the h-bank windows
            # (ty0-h0 needs rows<16, ty0-h1<52, ty1-h1<84, ty2-h1<116->all),
            # x1 tile-bands interleaved where the PE will need them
            ld_x1(nc.sync, 0, 0, 16 * 128)
            ld_x1(nc.scalar, 1, 0, 16 * 128)
            ld_x2(nc.sync, 0, 0, 16)
            ld_x2(nc.scalar, 1, 0, 16)
            ld_x2(nc.sync, 0, 16, 52)
            ld_x2(nc.scalar, 1, 16, 52)
            ld_x1(nc.sync, 0, 16 * 128, 32 * 128)
            ld_x1(nc.scalar, 1, 16 * 128, 32 * 128)
            ld_x2(nc.sync, 0, 52, 84)
            ld_x2(nc.scalar, 1, 52, 84)
            ld_x1(nc.sync, 0, 32 * 128, 64 * 128)
            ld_x1(nc.scalar, 1, 32 * 128, 64 * 128)
            ld_x2(nc.sync, 0, 84, H)
            ld_x2(nc.scalar, 1, 84, H)
            ld_x1(nc.sync, 0, 64 * 128, H * W)
            ld_x1(nc.scalar, 1, 64 * 128, H * W)

            def rhs_ap(k, h, ty, tx, py, px):
                off = MARG + (32 * ty + py + 36 * h) * W + (16 * tx + px - MAXD)
                v = p_sb[k][:]
                return AP(
                    tensor=v.tensor,
                    offset=off,
                    ap=[[PSZ, 128], [2 * W, 18], [2, 28]],
                )

            t_idx = 0
            for ty in range(3):
                for tx in range(8):
                    s2 = spool.tile([128, 4 * NF], bf16)
                    for py in range(2):
                        for px in range(2):
                            u = 2 * py + px
                            lhs = [
                                a_sb[k][:, t_idx * 128 : (t_idx + 1) * 128]
                                for k in range(2)
                            ]
                            t_idx += 1
                            ps = [
                                ppool.tile(
                                    [128, NF // 2], f32, name=f"ps{h}", tag=f"ps{h}"
                                )
                                for h in range(2)
                            ]
                            # bank h=0 fully accumulated first so its copy
                            # starts while h=1's x2 rows may still be landing
                            for h in range(2):
                                for k in range(2):
                                    nc.tensor.matmul(
                                        ps[h][:],
                                        lhs[k],
                                        rhs_ap(k, h, ty, tx, py, px),
                                        start=(k == 0),
                                        stop=(k == 1),
                                    )
                                if h == 0:
                                    nc.vector.tensor_copy(
                                        s2[:, NF * u : NF * u + NF // 2], ps[0][:]
                                    )
                            nc.scalar.copy(
                                s2[:, NF * u + NF // 2 : NF * u + NF], ps[1][:]
                            )
                    quad = t_idx // 4 - 1
                    sv = s2[:]
                    for g in range(4):
                        src = AP(
                            tensor=sv.tensor,
                            offset=32 * g * (4 * NF) + 112 * g,
                            ap=[[4 * NF, 32], [NF, 4], [1, 672]],
                        )
                        eng = nc.sync if g % 2 == 0 else nc.scalar
                        eng.dma_start(z[quad, g], src)

    _split_multiwait(nc)
    return nc


def prep_inputs(input1, input2):
    x1 = (input1 * np.float32(1.0 / C)).astype(ml_dtypes.bfloat16)
    x1 = np.ascontiguousarray(
        x1.reshape(B, C, 3, A_, 2, 8, B_, 2).transpose(0, 1, 2, 5, 4, 7, 3, 6)
    ).reshape(B, C, H, W)
    x2 = input2.astype(ml_dtypes.bfloat16)
    return [{"x1": x1[b], "x2": x2[b]} for b in range(N_CORES)]


def kernel(input1, input2):
    global _cached
    from concourse import bass_utils

    if _cached is None:
        _cached = _build()
    nc = _cached

    in_maps = prep_inputs(input1, input2)
    res = bass_utils.run_bass_kernel_spmd(nc, in_maps, core_ids=list(range(N_CORES)))
    Z = np.stack([res.results[b]["z"] for b in range(N_CORES)])
    # [b, ty, tx, g, ip(4), j(8), py, px, rr(24), s(28)]
    Zf = Z.astype(np.float32).reshape(B, 3, 8, 4, 4, 8, 2, 2, 24, 28)

    D = 21
    out = np.empty((B, D * D, H, W), np.float32)
    # out8[b, d, ty, i, py, tx, j, px]: y=32ty+2i+py, x=16tx+2j+px
    out8 = out.reshape(B, D * D, 3, 16, 2, 8, 8, 2)
    for i in range(16):
        g, ip = i // 4, i % 4
        for j in range(8):
            blk = Zf[:, :, :, g, ip, j, :, :, ip : ip + D, j : j + D]
            # blk: [B, ty, tx, py, px, dy, dx]
            out8[:, :, :, i, :, :, j, :] = blk.reshape(
                B, 3, 8, 2, 2, D * D
            ).transpose(0, 5, 1, 3, 2, 4)
    for dxi in range(D):
        ox = 2 * dxi - MAXD
        if ox < 0:
            out[:, dxi::D, :, 0:-ox] = 0.0
        elif ox > 0:
            out[:, dxi::D, :, W - ox : W] = 0.0
    return out
